# revision 49
# baseline (speedup 1.0000x reference)
"""Causal self-attention (B=4, T=2048, D=1024, H=16, hd=64) on 8 trn2 NeuronCores.

Sharding: data parallel over batch (4) x tensor parallel over heads (2 groups
of 8). Core c handles batch c//2 and heads (c%2)*8 .. (c%2)*8+8.
Wq/Wk/Wv are column-parallel by head group, Wo row-parallel; the pair of
cores sharing a batch produce partial outputs that are summed on the host.

On-device layout (per core) is fully "transposed": projections produce
Q^T, K^T [512, 2048] and V [2048, 512], scores are computed as
S^T = K Q^T (j=key on partitions, i=query on free dim), softmax uses
exp without max subtraction (scores are O(6) here), the denominator
comes for free from a ones-column appended to V, and attention output
O^T [hd, T] feeds the row-parallel out-projection directly as lhsT.

The V bias never touches the device: softmax rows sum to 1, so
attn @ (V + bv) = attn @ V + bv, and bv @ Wo folds into the host-side
output bias.

Head pairs share one [128, 1024] exp; their S^T matmuls row-pack onto
the PE concurrently (partition offsets 0/64). Score matmuls and exps are
trimmed to causally-valid columns. The per-chunk emission is
software-pipelined (S of tile jt+1 ahead of AV of tile jt in the PE
stream) so the PE never waits on the scalar engine's exp; AV-B lags
AV-A by two j-tiles so the psum-slot WAR against the previous head
pair's normalize never stalls the PE.
"""

import contextlib
import ctypes
import sys
import types

import numpy as np

B, T, D = 4, 2048, 1024
H_TOT, HD = 16, 64
SCALE = HD ** -0.5
P = 128
NH = 8            # heads per core
QD = NH * HD      # 512, projected dim per core
KT = D // P       # 8 contraction tiles for projections
MT = QD // P      # 4 qdim tiles
TT = T // P       # 16 token tiles
ACH = 512         # phase-A1 token chunk (Q/K); PSUM bank caps matmul N at 512
NACH = T // ACH   # 4
ICH = 512         # attention query chunk
NIC = T // ICH    # 4

_PROGRAM = None  # compiled program cache — build once per process


def _install_ntff_hook():
    """antenv.axon_hooks is missing in this image; recreate it so
    run_bass_kernel_spmd(trace=True) can profile. Harmless if unused."""
    if "antenv.axon_hooks" in sys.modules:
        return
    try:
        import antenv
    except ImportError:
        return
    mod = types.ModuleType("antenv.axon_hooks")
    _hook = [None]
    mod.set_axon_ntff_profile_hook = lambda h: _hook.__setitem__(0, h)
    mod.get_axon_ntff_profile_hook = lambda: _hook[0]
    antenv.axon_hooks = mod
    sys.modules["antenv.axon_hooks"] = mod
    try:
        lib = ctypes.CDLL("/opt/axon/libaxon_pjrt.so")
        if not hasattr(lib, "axon_start_nrt_profile"):
            return
        lib.axon_start_nrt_profile.argtypes = [
            ctypes.POINTER(ctypes.c_int64), ctypes.c_size_t]
        lib.axon_start_nrt_profile.restype = ctypes.c_int64
        lib.axon_stop_nrt_profile.argtypes = [ctypes.c_char_p]
        lib.axon_stop_nrt_profile.restype = ctypes.c_int64

        @contextlib.contextmanager
        def _hookfn(output_dir, device_ids):
            import jax
            jax.devices()
            if device_ids:
                ids = (ctypes.c_int64 * len(device_ids))(*device_ids)
                rc = lib.axon_start_nrt_profile(ids, len(device_ids))
            else:
                rc = lib.axon_start_nrt_profile(None, 0)
            if rc != 0:
                raise RuntimeError(f"axon_start_nrt_profile rc={rc}")
            try:
                yield
            finally:
                n = lib.axon_stop_nrt_profile(str(output_dir).encode())
                print(f"profile: {n} file(s) written to {output_dir}")

        mod.set_axon_ntff_profile_hook(_hookfn)
    except OSError:
        pass


def _build_program():
    from contextlib import ExitStack

    import concourse.tile as tile
    from concourse import bacc, mybir

    F32 = mybir.dt.float32
    BF16 = mybir.dt.bfloat16
    AF = mybir.ActivationFunctionType
    ALU = mybir.AluOpType

    nc = bacc.Bacc("TRN2", target_bir_lowering=False, debug=False,
                   num_devices=8)

    # all tensor inputs arrive pre-arranged in SBUF layout [128, k, n]
    # (host does the transpose) so every DMA is long contiguous runs
    xT_d = nc.dram_tensor("xT", [P, KT * T], BF16, kind="ExternalInput").ap()
    wq_d = nc.dram_tensor("wq", [P, KT * QD], BF16, kind="ExternalInput").ap()
    wk_d = nc.dram_tensor("wk", [P, KT * QD], BF16, kind="ExternalInput").ap()
    wv_d = nc.dram_tensor("wv", [P, KT * QD], BF16, kind="ExternalInput").ap()
    wo_d = nc.dram_tensor("wo", [P, MT * D], BF16, kind="ExternalInput").ap()
    bq_d = nc.dram_tensor("bq", [P, MT], F32, kind="ExternalInput").ap()
    bk_d = nc.dram_tensor("bk", [P, MT], F32, kind="ExternalInput").ap()
    msk_d = nc.dram_tensor("msk", [P, P], BF16, kind="ExternalInput").ap()
    idn_d = nc.dram_tensor("idn", [P, P], BF16, kind="ExternalInput").ap()
    neg_d = nc.dram_tensor("neg", [P, P], BF16, kind="ExternalInput").ap()
    out_d = nc.dram_tensor("out", [T, D], F32, kind="ExternalOutput").ap()

    xT_k = xT_d.rearrange("p (k t) -> p k t", k=KT)      # [128, 8, 2048]
    wq_k = wq_d.rearrange("p (k m) -> p k m", k=KT)      # [128, 8, 512]
    wk_k = wk_d.rearrange("p (k m) -> p k m", k=KT)
    wv_k = wv_d.rearrange("p (k m) -> p k m", k=KT)
    wo_k = wo_d.rearrange("p (k e) -> p k e", k=MT)      # [128, 4, 1024]

    with tile.TileContext(nc) as tc, ExitStack() as ctx:
        persist = ctx.enter_context(tc.tile_pool(name="persist", bufs=1))

        qt = [persist.tile([P, T], BF16, name=f"qt{i}") for i in range(MT)]
        kt_ = [persist.tile([P, T], BF16, name=f"kt{i}") for i in range(MT)]
        v3 = [persist.tile([P, NH, HD + 1], BF16, name=f"v3_{i}")
              for i in range(TT)]
        at = [persist.tile([P, T], BF16, name=f"at{i}") for i in range(MT)]
        xt_all = persist.tile([P, KT, T], BF16, name="xt")

        wq_sb = persist.tile([P, KT, QD], BF16, name="wq")
        wk_sb = persist.tile([P, KT, QD], BF16, name="wk")
        bq_sb = persist.tile([P, MT], F32, name="bq")
        bk_sb = persist.tile([P, MT], F32, name="bk")
        tri_sb = persist.tile([P, P], BF16, name="tri")
        idn_sb = persist.tile([P, P], BF16, name="idn")
        neg_sb = persist.tile([P, P], BF16, name="neg")
        wv_sb = persist.tile([P, KT, QD], BF16, name="wv")
        wo_sb = persist.tile([P, MT, D], BF16, name="wo")

        # startup DMAs split per k-tile and interleaved in the order phase
        # A1 consumes them, so they spread across DMA queues and the first
        # matmul only waits on ~384KB instead of ~3MB
        # the very first matmul needs only wq[:,0,0:128] and xt[:,0,0:512];
        # give those their own tiny DMAs so it starts ~1us earlier
        nc.sync.dma_start(wq_sb[:, 0, 0:P], wq_k[:, 0, 0:P])
        nc.sync.dma_start(xt_all[:, 0, 0:ACH], xT_k[:, 0, 0:ACH])
        nc.sync.dma_start(wq_sb[:, 0, P:QD], wq_k[:, 0, P:QD])
        nc.sync.dma_start(xt_all[:, 0, ACH:T // 2], xT_k[:, 0, ACH:T // 2])
        for k in range(1, KT):
            nc.sync.dma_start(wq_sb[:, k, :], wq_k[:, k, :])
            nc.sync.dma_start(xt_all[:, k, 0:T // 2], xT_k[:, k, 0:T // 2])
        for k in range(KT):
            nc.sync.dma_start(wk_sb[:, k, :], wk_k[:, k, :])
        nc.sync.dma_start(bq_sb[:], bq_d)
        nc.sync.dma_start(bk_sb[:], bk_d)
        nc.sync.dma_start(idn_sb[:], idn_d)
        nc.sync.dma_start(neg_sb[:], neg_d)
        for k in range(KT):
            nc.sync.dma_start(xt_all[:, k, T // 2:T], xT_k[:, k, T // 2:T])
        nc.sync.dma_start(tri_sb[:], msk_d)
        nc.sync.dma_start(wv_sb[:], wv_k)
        nc.sync.dma_start(wo_sb[:], wo_k)
        for tt in range(TT):
            nc.vector.memset(v3[tt][:, :, HD:HD + 1], 1.0)

        # ---- phase A1: Q^T, K^T projections --------------------------------
        with tc.tile_pool(name="pjps1", bufs=1, space="PSUM") as pjp:
            # chunk-inner so each weight tile is loaded into the PE once
            # and reused for two 512-column chunks; two half-passes so the
            # first matmuls only wait on the first half of the xT DMA
            for half in range(2):
                for mt in range(MT):
                    for w_sb, dst, b_sb in ((wq_sb, qt, bq_sb),
                                            (wk_sb, kt_, bk_sb)):
                        ps = [pjp.tile([P, ACH], F32, name="pj", bufs=8)
                              for _ in range(2)]
                        for k in range(KT):
                            for i, nch in enumerate((2 * half, 2 * half + 1)):
                                nc.tensor.matmul(
                                    ps[i][:],
                                    w_sb[:, k, mt * P:(mt + 1) * P],
                                    xt_all[:, k, nch * ACH:(nch + 1) * ACH],
                                    start=(k == 0), stop=(k == KT - 1))
                        for i, nch in enumerate((2 * half, 2 * half + 1)):
                            csl = slice(nch * ACH, (nch + 1) * ACH)
                            nc.vector.tensor_scalar_add(dst[mt][:, csl],
                                                        ps[i][:],
                                                        b_sb[:, mt:mt + 1])
            # first V tiles inside the A1 psum epoch: their psum slot's
            # previous user retired 8 allocations ago, so no pool-close
            # barrier stalls the PE at the A1->attention transition
            for tt in range(4):
                psv = pjp.tile([P, ACH], F32, name="pj", bufs=8)
                for k in range(KT):
                    nc.tensor.matmul(
                        psv[:], xt_all[:, k, tt * P:(tt + 1) * P],
                        wv_sb[:, k, :], start=(k == 0), stop=(k == KT - 1))
                nc.vector.tensor_copy(
                    v3[tt][:, :, 0:HD],
                    psv[:].rearrange("p (h d) -> p h d", d=HD))

        # ---- phases A2/B/C interleaved per query chunk ---------------------
        with tc.tile_pool(name="attnsb", bufs=1) as ap_, \
             tc.tile_pool(name="obp", bufs=8) as obp, \
             tc.tile_pool(name="attnps", bufs=1, space="PSUM") as sp:

            def emit_v_tile(tt):
                psv = sp.tile([P, QD], F32, name="misc", bufs=1)
                for k in range(KT):
                    nc.tensor.matmul(
                        psv[:], xt_all[:, k, tt * P:(tt + 1) * P],
                        wv_sb[:, k, :], start=(k == 0), stop=(k == KT - 1))
                # tiles 4..11 are emitted inside the PE-bound chunks 0-1
                # where the scalar engine has slack: copying there keeps
                # the DVE queue free of 600ns bubbles in front of the
                # masks that gate AV
                eng = nc.scalar if 4 <= tt < 12 else nc.vector
                if eng is nc.scalar:
                    nc.scalar.copy(
                        v3[tt][:, :, 0:HD],
                        psv[:].rearrange("p (h d) -> p h d", d=HD))
                else:
                    nc.vector.tensor_copy(
                        v3[tt][:, :, 0:HD],
                        psv[:].rearrange("p (h d) -> p h d", d=HD))

            def emit_attn_chunk(ic, fillers=()):
                """Attention for query chunk ic, as 4 head-pairs, with the
                PE stream software-pipelined: S of j-tile jt+1 is emitted
                before AV of j-tile jt so exp latency is hidden, and the
                AV-B stream lags AV-A by four j-tiles so the opsum-slot
                WAR against the previous head pair's normalize clears
                before the PE reaches the first AV-B. The normalize is
                staged: reciprocal+broadcast are emitted as soon as each
                ops accumulation stops (they overlap the AV tail), and the
                final multiplies are deferred past the next head pair's
                first masks so they never block the DVE ops that feed the
                PE. One filler (a V-tile projection or an out-projection
                group for another chunk) is emitted per head-pair boundary
                to keep the PE fed while the scalar engine works through
                the exps."""
                isl = slice(ic * ICH, (ic + 1) * ICH)
                njt = 4 * ic + 4
                fillers = list(fillers)
                # spread fillers evenly over head pairs: the scalar engine's
                # per-j-tile deficit is uniform, so front-loading them leaves
                # the last head pairs exp-starved
                quota = -(-len(fillers) // MT)
                pending_mults = []
                for hp in range(MT):
                    popped = 0
                    # allocated full-bank so the final out-projection waves
                    # can reuse freed opsum slots with an identical tag shape
                    opsA = sp.tile([P, ICH], F32, name="opsum",
                                   bufs=3)[0:HD + 1, :]
                    opsB = sp.tile([P, ICH], F32, name="opsum",
                                   bufs=3)[0:HD + 1, :]
                    s2s, e2s = {}, {}

                    def emit_s(jt):
                        # columns left of the diagonal block are causally
                        # invalid — skip them in the score matmuls
                        kdiag = jt - 4 * ic
                        c0 = max(kdiag, 0) * P
                        s2 = sp.tile([P, 2 * ICH], F32, name="spsum", bufs=2)
                        jsl = slice(jt * P, (jt + 1) * P)
                        qsl = slice(ic * ICH + c0, (ic + 1) * ICH)
                        nc.tensor.matmul(s2[:, c0:ICH], kt_[hp][0:HD, jsl],
                                         qt[hp][0:HD, qsl],
                                         start=True, stop=True)
                        nc.tensor.matmul(s2[:, ICH + c0:2 * ICH],
                                         kt_[hp][HD:P, jsl],
                                         qt[hp][HD:P, qsl],
                                         start=True, stop=True)
                        if ic == 0:
                            # ic0 is DVE-chain-bound: mask the diagonal
                            # block on the PE instead, as an additive -1e5
                            # accumulated onto the scores pre-exp
                            for o in (c0, ICH + c0):
                                nc.tensor.matmul(
                                    s2[:, o:o + P], idn_sb[:], neg_sb[:],
                                    start=False, stop=True,
                                    skip_group_check=True)
                        s2s[jt] = s2

                    def emit_exp(jt):
                        kdiag = jt - 4 * ic
                        c0 = max(kdiag, 0) * P
                        e2 = ap_.tile([P, 2 * ICH], BF16, name="e", bufs=5)
                        s2 = s2s.pop(jt)
                        if ic <= 1:
                            # two half-activations: halves the exp->AV
                            # latency in the shallow-pipeline chunks where
                            # the scalar engine has slack
                            nc.scalar.activation(e2[:, c0:ICH],
                                                 s2[:, c0:ICH], AF.Exp)
                            nc.scalar.activation(e2[:, ICH + c0:2 * ICH],
                                                 s2[:, ICH + c0:2 * ICH],
                                                 AF.Exp)
                        else:
                            # one activation over the contiguous valid
                            # span; the dead middle [ICH, ICH+c0) is
                            # computed on stale psum but never read
                            nc.scalar.activation(e2[:, c0:2 * ICH],
                                                 s2[:, c0:2 * ICH],
                                                 AF.Exp)
                        if kdiag >= 0 and ic != 0:
                            # zero the diagonal block's upper triangle
                            for half in range(2):
                                o = half * ICH + c0
                                nc.vector.tensor_tensor(
                                    e2[:, o:o + P], e2[:, o:o + P],
                                    tri_sb[:], op=ALU.mult)
                        e2s[jt] = e2

                    def emit_av_a(jt):
                        kdiag = jt - 4 * ic
                        c0 = max(kdiag, 0) * P
                        nc.tensor.matmul(opsA[:, c0:],
                                         v3[jt][:, 2 * hp, :],
                                         e2s[jt][:, c0:ICH],
                                         start=(jt == 0),
                                         stop=(jt == njt - 1))

                    def emit_av_b(jt):
                        kdiag = jt - 4 * ic
                        c0 = max(kdiag, 0) * P
                        e2 = e2s.pop(jt)
                        nc.tensor.matmul(opsB[:, c0:],
                                         v3[jt][:, 2 * hp + 1, :],
                                         e2[:, ICH + c0:2 * ICH],
                                         start=(jt == 0),
                                         stop=(jt == njt - 1))

                    def norm_pre(ops):
                        # reciprocal of the ones-column denominator row,
                        # broadcast across the head dim; runs while the PE
                        # is still draining the AV tail. (The copy is
                        # needed: reciprocal_approx_fast misreads a PSUM
                        # source at a nonzero base partition.)
                        dn = ap_.tile([1, ICH], F32, name="dn", bufs=4)
                        nc.vector.tensor_copy(dn[:], ops[HD:HD + 1, :])
                        recip = ap_.tile([1, ICH], F32, name="recip", bufs=4)
                        nc.vector.reciprocal_approx_fast(recip[:], dn[:])
                        rb = ap_.tile([HD, ICH], F32, name="rb", bufs=4)
                        nc.gpsimd.partition_broadcast(rb[:], recip[:])
                        return rb

                    emit_s(0)
                    for jt in range(1, njt):
                        emit_s(jt)
                        site = jt % 5 == 4 or (njt <= 8 and jt == 2)
                        if site and fillers and popped < quota:
                            # mid-pair PE filler: the scalar engine's exp
                            # throughput trails the PE by ~200ns per j-tile
                            fillers.pop(0)()
                            popped += 1
                        emit_exp(jt - 1)
                        if jt == 1:
                            while pending_mults:
                                pending_mults.pop()()
                        emit_av_a(jt - 1)
                        if jt >= 4:
                            emit_av_b(jt - 4)
                    emit_exp(njt - 1)
                    if fillers and (popped < quota or hp == MT - 1):
                        # hide the final exp's latency behind independent work
                        fillers.pop(0)()
                    emit_av_a(njt - 1)
                    last = ic == NIC - 1 and hp == MT - 1
                    if not last:
                        rbA = norm_pre(opsA)
                        for jt in range(max(njt - 4, 0), njt):
                            emit_av_b(jt)
                        rbB = norm_pre(opsB)
                    else:
                        # very last head pair: the whole kernel tail waits
                        # on this chain, so the denominator copies go on
                        # the (idle) scalar engine and the broadcasts are
                        # split by column half so the first normalize
                        # pieces land as early as possible
                        H2 = ICH // 2
                        dnA = ap_.tile([1, ICH], F32, name="dn", bufs=4)
                        nc.scalar.copy(dnA[:], opsA[HD:HD + 1, :])
                        rcA = ap_.tile([1, ICH], F32, name="recip", bufs=4)
                        nc.vector.reciprocal_approx_fast(rcA[:], dnA[:])
                        rbA = ap_.tile([HD, ICH], F32, name="rb", bufs=4)
                        nc.gpsimd.partition_broadcast(rbA[:, 0:H2],
                                                      rcA[:, 0:H2])
                        for jt in range(max(njt - 4, 0), njt):
                            emit_av_b(jt)
                        dnB = ap_.tile([1, ICH], F32, name="dn", bufs=4)
                        nc.scalar.copy(dnB[:], opsB[HD:HD + 1, :])
                        rcB = ap_.tile([1, ICH], F32, name="recip", bufs=4)
                        nc.vector.reciprocal_approx_fast(rcB[:], dnB[:])
                        rbB = ap_.tile([HD, ICH], F32, name="rb", bufs=4)
                        nc.gpsimd.partition_broadcast(rbB[:, 0:H2],
                                                      rcB[:, 0:H2])
                        nc.gpsimd.partition_broadcast(rbA[:, H2:ICH],
                                                      rcA[:, H2:ICH])
                        nc.gpsimd.partition_broadcast(rbB[:, H2:ICH],
                                                      rcB[:, H2:ICH])

                    def norm_mult(split=False, hp=hp, opsA=opsA, opsB=opsB,
                                  rbA=rbA, rbB=rbB):
                        # normalize straight out of PSUM: in0 is PSUM so the
                        # SBUF base-partition pairing rule doesn't apply.
                        # split=True (very last head pair) emits 128-column
                        # pieces so the final out-projection's k=3 matmuls
                        # unblock progressively instead of all at once.
                        pieces = range(4) if split else (slice(None),)
                        for pc in pieces:
                            csl = (slice(pc * P, (pc + 1) * P)
                                   if isinstance(pc, int) else pc)
                            asl = slice(ic * ICH + (csl.start or 0),
                                        ic * ICH + (csl.stop or ICH))
                            for po, ops, rb in ((0, opsA, rbA),
                                                (HD, opsB, rbB)):
                                nc.vector.tensor_tensor(
                                    at[hp][po:po + HD, asl],
                                    ops[0:HD, csl], rb[:, csl],
                                    op=ALU.mult)

                    pending_mults.append(norm_mult)

                while pending_mults:
                    pending_mults.pop()(split=(ic == NIC - 1))
                for f in fillers:
                    f()

            def emit_out_group(mt, nch2):
                pso = sp.tile([P, 512], F32, name="misc", bufs=1)
                for k in range(MT):
                    nc.tensor.matmul(
                        pso[:], at[k][:, mt * P:(mt + 1) * P],
                        wo_sb[:, k, nch2 * 512:(nch2 + 1) * 512],
                        start=(k == 0), stop=(k == MT - 1))
                ob = obp.tile([P, 512], F32, name="ob")
                nc.vector.tensor_copy(ob[:], pso[:])
                nc.sync.dma_start(
                    out_d[mt * P:(mt + 1) * P,
                          nch2 * 512:(nch2 + 1) * 512], ob[:])

            for ic in range(NIC):
                fillers = []
                if ic + 1 < NIC:
                    fillers += [
                        (lambda tt=tt: emit_v_tile(tt))
                        for tt in range(4 * ic + 4, 4 * ic + 8)]
                if ic > 0:
                    fillers += [
                        (lambda mt=mt, n=n: emit_out_group(mt, n))
                        for mt in range(4 * (ic - 1), 4 * ic)
                        for n in range(2)]
                emit_attn_chunk(ic, fillers)

            # final out-projection: all eight groups (mt 12..15 x both
            # halves) live on eight distinct psum banks (four spsum
            # halves, the three opsum slots, and misc) so no slot-reuse
            # WAR can stall the PE. Six groups launch k-major (eighteen
            # ready matmuls cover the last normalize chain); their k=3
            # matmuls follow in mt order, matching the column-split
            # normalize so each unblocks as its at-piece lands. The last
            # two groups sit on the opsum slots of the final head pair
            # and start once its normalize mults have read them. Copies
            # alternate between the now-idle scalar and vector engines.
            groups6 = [(12, 0), (12, 1), (13, 0), (13, 1), (14, 0), (15, 0)]
            slots = []
            for _ in range(2):
                t = sp.tile([P, 2 * ICH], F32, name="spsum", bufs=2)
                slots += [t[:, 0:512], t[:, 512:1024]]
            slots.insert(1, sp.tile([P, ICH], F32, name="opsum", bufs=3))
            slots.insert(3, sp.tile([P, 512], F32, name="misc", bufs=1))
            for k in range(MT - 1):
                for g, (mt, nch2) in enumerate(groups6):
                    nc.tensor.matmul(
                        slots[g], at[k][:, mt * P:(mt + 1) * P],
                        wo_sb[:, k, nch2 * 512:(nch2 + 1) * 512],
                        start=(k == 0), stop=False)
            for g, (mt, nch2) in enumerate(groups6):
                nc.tensor.matmul(
                    slots[g], at[MT - 1][:, mt * P:(mt + 1) * P],
                    wo_sb[:, MT - 1, nch2 * 512:(nch2 + 1) * 512],
                    start=False, stop=True)
            tail2 = [(14, 1), (15, 1)]
            slots2 = [sp.tile([P, ICH], F32, name="opsum", bufs=3)
                      for _ in tail2]
            for k in range(MT):
                for g, (mt, nch2) in enumerate(tail2):
                    nc.tensor.matmul(
                        slots2[g], at[k][:, mt * P:(mt + 1) * P],
                        wo_sb[:, k, nch2 * 512:(nch2 + 1) * 512],
                        start=(k == 0), stop=(k == MT - 1))
            # groups6 copies go on scalar only: a vector copy here would
            # queue in front of the normalize piece-mults on the DVE and
            # delay the tail groups behind them
            for g, (mt, nch2) in enumerate(groups6 + tail2):
                pso = (slots + slots2)[g]
                ob = obp.tile([P, 512], F32, name="ob")
                if g < len(groups6):
                    nc.scalar.copy(ob[:], pso)
                else:
                    nc.vector.tensor_copy(ob[:], pso)
                nc.sync.dma_start(
                    out_d[mt * P:(mt + 1) * P,
                          nch2 * 512:(nch2 + 1) * 512], ob[:])

    nc.compile()
    return nc


def _get_program():
    global _PROGRAM
    if _PROGRAM is None:
        _install_ntff_hook()
        _PROGRAM = _build_program()
    return _PROGRAM


def _make_masks():
    """Multiplicative upper-triangle zero mask [128, 128] for the diagonal
    128x128 block of each S^T tile: entry (j, i) = 1 if j <= i else 0."""
    j = np.arange(P)[:, None]
    i = np.arange(P)[None, :]
    return (j <= i).astype(np.float32)


def make_in_maps(x, Wq, bq, Wk, bk, Wv, bv, Wo, bo):
    import ml_dtypes
    bf16 = ml_dtypes.bfloat16

    def sbl(a, k):
        """[k*128, n] -> SBUF layout [128, k*n] (partition-major runs)."""
        n = a.shape[1]
        return np.ascontiguousarray(
            a.reshape(k, P, n).transpose(1, 0, 2).reshape(P, k * n)
        ).astype(bf16)

    masks = _make_masks()
    in_maps = []
    for c in range(8):
        b, hg = c // 2, c % 2
        sl = slice(hg * QD, (hg + 1) * QD)
        in_maps.append({
            "xT": sbl(np.ascontiguousarray(x[b].T), KT),
            "wq": sbl(Wq[:, sl] * SCALE, KT),
            "wk": sbl(Wk[:, sl], KT),
            "wv": sbl(Wv[:, sl], KT),
            "wo": sbl(Wo[sl, :], MT),
            "bq": np.ascontiguousarray((bq[sl] * SCALE).reshape(MT, P).T),
            "bk": np.ascontiguousarray(bk[sl].reshape(MT, P).T),
            "msk": masks.astype(bf16),
            "idn": np.eye(P, dtype=np.float32).astype(bf16),
            "neg": ((1.0 - masks) * -100000.0).astype(bf16),
        })
    return in_maps


def run(inputs, trace=False):
    from concourse.bass_utils import run_bass_kernel_spmd

    nc = _get_program()
    in_maps = make_in_maps(**inputs)
    res = run_bass_kernel_spmd(nc, in_maps, list(range(8)), trace=trace)
    # softmax rows sum to 1, so the V bias adds bv to every attention
    # output exactly; fold bv @ Wo into the host-side output bias
    bo_eff = inputs["bo"] + inputs["bv"].astype(np.float64) @ \
        inputs["Wo"].astype(np.float64)
    bo_eff = bo_eff.astype(np.float32)
    out = np.empty((B, T, D), dtype=np.float32)
    for b in range(B):
        out[b] = res.results[2 * b]["out"] + res.results[2 * b + 1]["out"] \
            + bo_eff
    return out, res


def kernel(**inputs):
    inputs = {k: np.asarray(v) for k, v in inputs.items()}
    out, _ = run(inputs)
    return out


# revision 52
# speedup vs baseline: 1.0039x; 1.0039x over previous
"""Causal self-attention (B=4, T=2048, D=1024, H=16, hd=64) on 8 trn2 NeuronCores.

Sharding: data parallel over batch (4) x tensor parallel over heads (2 groups
of 8). Core c handles batch c//2 and heads (c%2)*8 .. (c%2)*8+8.
Wq/Wk/Wv are column-parallel by head group, Wo row-parallel; the pair of
cores sharing a batch produce partial outputs that are summed on the host.

On-device layout (per core) is fully "transposed": projections produce
Q^T, K^T [512, 2048] and V [2048, 512], scores are computed as
S^T = K Q^T (j=key on partitions, i=query on free dim), softmax uses
exp without max subtraction (scores are O(6) here), the denominator
comes for free from a ones-column appended to V, and attention output
O^T [hd, T] feeds the row-parallel out-projection directly as lhsT.

The V bias never touches the device: softmax rows sum to 1, so
attn @ (V + bv) = attn @ V + bv, and bv @ Wo folds into the host-side
output bias.

Head pairs share one [128, 1024] exp; their S^T matmuls row-pack onto
the PE concurrently (partition offsets 0/64). Score matmuls and exps are
trimmed to causally-valid columns. The per-chunk emission is
software-pipelined (S of tile jt+1 ahead of AV of tile jt in the PE
stream) so the PE never waits on the scalar engine's exp; AV-B lags
AV-A by two j-tiles so the psum-slot WAR against the previous head
pair's normalize never stalls the PE.
"""

import contextlib
import ctypes
import sys
import types

import numpy as np

B, T, D = 4, 2048, 1024
H_TOT, HD = 16, 64
SCALE = HD ** -0.5
P = 128
NH = 8            # heads per core
QD = NH * HD      # 512, projected dim per core
KT = D // P       # 8 contraction tiles for projections
MT = QD // P      # 4 qdim tiles
TT = T // P       # 16 token tiles
ACH = 512         # phase-A1 token chunk (Q/K); PSUM bank caps matmul N at 512
NACH = T // ACH   # 4
ICH = 512         # attention query chunk
NIC = T // ICH    # 4

_PROGRAM = None  # compiled program cache — build once per process


def _install_ntff_hook():
    """antenv.axon_hooks is missing in this image; recreate it so
    run_bass_kernel_spmd(trace=True) can profile. Harmless if unused."""
    if "antenv.axon_hooks" in sys.modules:
        return
    try:
        import antenv
    except ImportError:
        return
    mod = types.ModuleType("antenv.axon_hooks")
    _hook = [None]
    mod.set_axon_ntff_profile_hook = lambda h: _hook.__setitem__(0, h)
    mod.get_axon_ntff_profile_hook = lambda: _hook[0]
    antenv.axon_hooks = mod
    sys.modules["antenv.axon_hooks"] = mod
    try:
        lib = ctypes.CDLL("/opt/axon/libaxon_pjrt.so")
        if not hasattr(lib, "axon_start_nrt_profile"):
            return
        lib.axon_start_nrt_profile.argtypes = [
            ctypes.POINTER(ctypes.c_int64), ctypes.c_size_t]
        lib.axon_start_nrt_profile.restype = ctypes.c_int64
        lib.axon_stop_nrt_profile.argtypes = [ctypes.c_char_p]
        lib.axon_stop_nrt_profile.restype = ctypes.c_int64

        @contextlib.contextmanager
        def _hookfn(output_dir, device_ids):
            import jax
            jax.devices()
            if device_ids:
                ids = (ctypes.c_int64 * len(device_ids))(*device_ids)
                rc = lib.axon_start_nrt_profile(ids, len(device_ids))
            else:
                rc = lib.axon_start_nrt_profile(None, 0)
            if rc != 0:
                raise RuntimeError(f"axon_start_nrt_profile rc={rc}")
            try:
                yield
            finally:
                n = lib.axon_stop_nrt_profile(str(output_dir).encode())
                print(f"profile: {n} file(s) written to {output_dir}")

        mod.set_axon_ntff_profile_hook(_hookfn)
    except OSError:
        pass


def _build_program():
    from contextlib import ExitStack

    import concourse.tile as tile
    from concourse import bacc, mybir

    F32 = mybir.dt.float32
    BF16 = mybir.dt.bfloat16
    AF = mybir.ActivationFunctionType
    ALU = mybir.AluOpType

    nc = bacc.Bacc("TRN2", target_bir_lowering=False, debug=False,
                   num_devices=8)

    # all tensor inputs arrive pre-arranged in SBUF layout [128, k, n]
    # (host does the transpose) so every DMA is long contiguous runs
    xT_d = nc.dram_tensor("xT", [P, KT * T], BF16, kind="ExternalInput").ap()
    wq_d = nc.dram_tensor("wq", [P, KT * QD], BF16, kind="ExternalInput").ap()
    wk_d = nc.dram_tensor("wk", [P, KT * QD], BF16, kind="ExternalInput").ap()
    wv_d = nc.dram_tensor("wv", [P, KT * QD], BF16, kind="ExternalInput").ap()
    wo_d = nc.dram_tensor("wo", [P, MT * D], BF16, kind="ExternalInput").ap()
    bq_d = nc.dram_tensor("bq", [P, MT], F32, kind="ExternalInput").ap()
    bk_d = nc.dram_tensor("bk", [P, MT], F32, kind="ExternalInput").ap()
    msk_d = nc.dram_tensor("msk", [P, P], BF16, kind="ExternalInput").ap()
    idn_d = nc.dram_tensor("idn", [P, P], BF16, kind="ExternalInput").ap()
    neg_d = nc.dram_tensor("neg", [P, P], BF16, kind="ExternalInput").ap()
    out_d = nc.dram_tensor("out", [T, D], F32, kind="ExternalOutput").ap()

    xT_k = xT_d.rearrange("p (k t) -> p k t", k=KT)      # [128, 8, 2048]
    wq_k = wq_d.rearrange("p (k m) -> p k m", k=KT)      # [128, 8, 512]
    wk_k = wk_d.rearrange("p (k m) -> p k m", k=KT)
    wv_k = wv_d.rearrange("p (k m) -> p k m", k=KT)
    wo_k = wo_d.rearrange("p (k e) -> p k e", k=MT)      # [128, 4, 1024]

    with tile.TileContext(nc) as tc, ExitStack() as ctx:
        persist = ctx.enter_context(tc.tile_pool(name="persist", bufs=1))

        qt = [persist.tile([P, T], BF16, name=f"qt{i}") for i in range(MT)]
        kt_ = [persist.tile([P, T], BF16, name=f"kt{i}") for i in range(MT)]
        v3 = [persist.tile([P, NH, HD + 1], BF16, name=f"v3_{i}")
              for i in range(TT)]
        at = [persist.tile([P, T], BF16, name=f"at{i}") for i in range(MT)]
        xt_all = persist.tile([P, KT, T], BF16, name="xt")

        wq_sb = persist.tile([P, KT, QD], BF16, name="wq")
        wk_sb = persist.tile([P, KT, QD], BF16, name="wk")
        bq_sb = persist.tile([P, MT], F32, name="bq")
        bk_sb = persist.tile([P, MT], F32, name="bk")
        tri_sb = persist.tile([P, P], BF16, name="tri")
        idn_sb = persist.tile([P, P], BF16, name="idn")
        neg_sb = persist.tile([P, P], BF16, name="neg")
        wv_sb = persist.tile([P, KT, QD], BF16, name="wv")
        wo_sb = persist.tile([P, MT, D], BF16, name="wo")

        # startup DMAs split per k-tile and interleaved in the order phase
        # A1 consumes them, so they spread across DMA queues and the first
        # matmul only waits on ~384KB instead of ~3MB
        # the very first matmul needs only wq[:,0,0:128] and xt[:,0,0:512];
        # give those their own tiny DMAs so it starts ~1us earlier
        nc.sync.dma_start(wq_sb[:, 0, 0:P], wq_k[:, 0, 0:P])
        nc.sync.dma_start(xt_all[:, 0, 0:ACH], xT_k[:, 0, 0:ACH])
        nc.sync.dma_start(wq_sb[:, 0, P:QD], wq_k[:, 0, P:QD])
        nc.sync.dma_start(xt_all[:, 0, ACH:T // 2], xT_k[:, 0, ACH:T // 2])
        for k in range(1, KT):
            nc.sync.dma_start(wq_sb[:, k, :], wq_k[:, k, :])
            nc.sync.dma_start(xt_all[:, k, 0:T // 2], xT_k[:, k, 0:T // 2])
        for k in range(KT):
            nc.sync.dma_start(wk_sb[:, k, :], wk_k[:, k, :])
        nc.sync.dma_start(bq_sb[:], bq_d)
        nc.sync.dma_start(bk_sb[:], bk_d)
        nc.sync.dma_start(idn_sb[:], idn_d)
        nc.sync.dma_start(neg_sb[:], neg_d)
        for k in range(KT):
            nc.sync.dma_start(xt_all[:, k, T // 2:T], xT_k[:, k, T // 2:T])
        nc.sync.dma_start(tri_sb[:], msk_d)
        nc.sync.dma_start(wv_sb[:], wv_k)
        nc.sync.dma_start(wo_sb[:], wo_k)
        for tt in range(TT):
            nc.vector.memset(v3[tt][:, :, HD:HD + 1], 1.0)

        # ---- phase A1: Q^T, K^T projections --------------------------------
        with tc.tile_pool(name="pjps1", bufs=1, space="PSUM") as pjp:
            # chunk-inner so each weight tile is loaded into the PE once
            # and reused for two 512-column chunks; two half-passes so the
            # first matmuls only wait on the first half of the xT DMA
            for half in range(2):
                for mt in range(MT):
                    for w_sb, dst, b_sb in ((wq_sb, qt, bq_sb),
                                            (wk_sb, kt_, bk_sb)):
                        ps = [pjp.tile([P, ACH], F32, name="pj", bufs=8)
                              for _ in range(2)]
                        for k in range(KT):
                            for i, nch in enumerate((2 * half, 2 * half + 1)):
                                nc.tensor.matmul(
                                    ps[i][:],
                                    w_sb[:, k, mt * P:(mt + 1) * P],
                                    xt_all[:, k, nch * ACH:(nch + 1) * ACH],
                                    start=(k == 0), stop=(k == KT - 1))
                        for i, nch in enumerate((2 * half, 2 * half + 1)):
                            csl = slice(nch * ACH, (nch + 1) * ACH)
                            nc.vector.tensor_scalar_add(dst[mt][:, csl],
                                                        ps[i][:],
                                                        b_sb[:, mt:mt + 1])
            # first V tiles inside the A1 psum epoch: their psum slot's
            # previous user retired 8 allocations ago, so no pool-close
            # barrier stalls the PE at the A1->attention transition
            for tt in range(4):
                psv = pjp.tile([P, ACH], F32, name="pj", bufs=8)
                for k in range(KT):
                    nc.tensor.matmul(
                        psv[:], xt_all[:, k, tt * P:(tt + 1) * P],
                        wv_sb[:, k, :], start=(k == 0), stop=(k == KT - 1))
                nc.vector.tensor_copy(
                    v3[tt][:, :, 0:HD],
                    psv[:].rearrange("p (h d) -> p h d", d=HD))

        # ---- phases A2/B/C interleaved per query chunk ---------------------
        with tc.tile_pool(name="attnsb", bufs=1) as ap_, \
             tc.tile_pool(name="obp", bufs=8) as obp, \
             tc.tile_pool(name="attnps", bufs=1, space="PSUM") as sp:

            def emit_v_tile(tt):
                psv = sp.tile([P, QD], F32, name="misc", bufs=1)
                for k in range(KT):
                    nc.tensor.matmul(
                        psv[:], xt_all[:, k, tt * P:(tt + 1) * P],
                        wv_sb[:, k, :], start=(k == 0), stop=(k == KT - 1))
                nc.vector.tensor_copy(
                    v3[tt][:, :, 0:HD],
                    psv[:].rearrange("p (h d) -> p h d", d=HD))

            def emit_attn_chunk(ic, fillers=()):
                """Attention for query chunk ic, as 4 head-pairs, with the
                PE stream software-pipelined: S of j-tile jt+1 is emitted
                before AV of j-tile jt so exp latency is hidden, and the
                AV-B stream lags AV-A by four j-tiles so the opsum-slot
                WAR against the previous head pair's normalize clears
                before the PE reaches the first AV-B. The normalize is
                staged: reciprocal+broadcast are emitted as soon as each
                ops accumulation stops (they overlap the AV tail), and the
                final multiplies are deferred past the next head pair's
                first masks so they never block the DVE ops that feed the
                PE. One filler (a V-tile projection or an out-projection
                group for another chunk) is emitted per head-pair boundary
                to keep the PE fed while the scalar engine works through
                the exps."""
                isl = slice(ic * ICH, (ic + 1) * ICH)
                njt = 4 * ic + 4
                fillers = list(fillers)
                # spread fillers evenly over head pairs: the scalar engine's
                # per-j-tile deficit is uniform, so front-loading them leaves
                # the last head pairs exp-starved
                quota = -(-len(fillers) // MT)
                pending_mults = []
                for hp in range(MT):
                    popped = 0
                    # allocated full-bank so the final out-projection waves
                    # can reuse freed opsum slots with an identical tag shape
                    opsA = sp.tile([P, ICH], F32, name="opsum",
                                   bufs=3)[0:HD + 1, :]
                    opsB = sp.tile([P, ICH], F32, name="opsum",
                                   bufs=3)[0:HD + 1, :]
                    s2s, e2s = {}, {}

                    def emit_s(jt):
                        # columns left of the diagonal block are causally
                        # invalid — skip them in the score matmuls
                        kdiag = jt - 4 * ic
                        c0 = max(kdiag, 0) * P
                        s2 = sp.tile([P, 2 * ICH], F32, name="spsum", bufs=2)
                        jsl = slice(jt * P, (jt + 1) * P)
                        qsl = slice(ic * ICH + c0, (ic + 1) * ICH)
                        nc.tensor.matmul(s2[:, c0:ICH], kt_[hp][0:HD, jsl],
                                         qt[hp][0:HD, qsl],
                                         start=True, stop=True)
                        nc.tensor.matmul(s2[:, ICH + c0:2 * ICH],
                                         kt_[hp][HD:P, jsl],
                                         qt[hp][HD:P, qsl],
                                         start=True, stop=True)
                        if ic == 0:
                            # ic0 is DVE-chain-bound: mask the diagonal
                            # block on the PE instead, as an additive -1e5
                            # accumulated onto the scores pre-exp
                            for o in (c0, ICH + c0):
                                nc.tensor.matmul(
                                    s2[:, o:o + P], idn_sb[:], neg_sb[:],
                                    start=False, stop=True,
                                    skip_group_check=True)
                        s2s[jt] = s2

                    def emit_exp(jt):
                        kdiag = jt - 4 * ic
                        c0 = max(kdiag, 0) * P
                        e2 = ap_.tile([P, 2 * ICH], BF16, name="e", bufs=6)
                        s2 = s2s.pop(jt)
                        if ic == 0:
                            # two half-activations: halves the exp->AV
                            # latency in the shallow-pipeline chunks where
                            # the scalar engine has slack
                            nc.scalar.activation(e2[:, c0:ICH],
                                                 s2[:, c0:ICH], AF.Exp)
                            nc.scalar.activation(e2[:, ICH + c0:2 * ICH],
                                                 s2[:, ICH + c0:2 * ICH],
                                                 AF.Exp)
                        else:
                            # one activation over the contiguous valid
                            # span; the dead middle [ICH, ICH+c0) is
                            # computed on stale psum but never read
                            nc.scalar.activation(e2[:, c0:2 * ICH],
                                                 s2[:, c0:2 * ICH],
                                                 AF.Exp)
                        if kdiag >= 0 and ic != 0:
                            # zero the diagonal block's upper triangle
                            for half in range(2):
                                o = half * ICH + c0
                                nc.vector.tensor_tensor(
                                    e2[:, o:o + P], e2[:, o:o + P],
                                    tri_sb[:], op=ALU.mult)
                        e2s[jt] = e2

                    def emit_av_a(jt):
                        kdiag = jt - 4 * ic
                        c0 = max(kdiag, 0) * P
                        nc.tensor.matmul(opsA[:, c0:],
                                         v3[jt][:, 2 * hp, :],
                                         e2s[jt][:, c0:ICH],
                                         start=(jt == 0),
                                         stop=(jt == njt - 1))

                    def emit_av_b(jt):
                        kdiag = jt - 4 * ic
                        c0 = max(kdiag, 0) * P
                        e2 = e2s.pop(jt)
                        nc.tensor.matmul(opsB[:, c0:],
                                         v3[jt][:, 2 * hp + 1, :],
                                         e2[:, ICH + c0:2 * ICH],
                                         start=(jt == 0),
                                         stop=(jt == njt - 1))

                    def norm_pre(ops):
                        # reciprocal of the ones-column denominator row,
                        # broadcast across the head dim; runs while the PE
                        # is still draining the AV tail. (The copy is
                        # needed: reciprocal_approx_fast misreads a PSUM
                        # source at a nonzero base partition.)
                        dn = ap_.tile([1, ICH], F32, name="dn", bufs=4)
                        nc.vector.tensor_copy(dn[:], ops[HD:HD + 1, :])
                        recip = ap_.tile([1, ICH], F32, name="recip", bufs=4)
                        nc.vector.reciprocal_approx_fast(recip[:], dn[:])
                        rb = ap_.tile([HD, ICH], F32, name="rb", bufs=4)
                        nc.gpsimd.partition_broadcast(rb[:], recip[:])
                        return rb

                    emit_s(0)
                    for jt in range(1, njt):
                        emit_s(jt)
                        site = jt % 5 == 4 or (njt <= 8 and jt == 2)
                        if site and fillers and popped < quota:
                            # mid-pair PE filler: the scalar engine's exp
                            # throughput trails the PE by ~200ns per j-tile
                            fillers.pop(0)()
                            popped += 1
                        emit_exp(jt - 1)
                        if jt == 1:
                            while pending_mults:
                                pending_mults.pop()()
                        emit_av_a(jt - 1)
                        if jt >= 4:
                            emit_av_b(jt - 4)
                    emit_exp(njt - 1)
                    if fillers and (popped < quota or hp == MT - 1):
                        # hide the final exp's latency behind independent work
                        fillers.pop(0)()
                    emit_av_a(njt - 1)
                    last = ic == NIC - 1 and hp == MT - 1
                    if not last:
                        rbA = norm_pre(opsA)
                        for jt in range(max(njt - 4, 0), njt):
                            emit_av_b(jt)
                        rbB = norm_pre(opsB)
                    else:
                        # very last head pair: the whole kernel tail waits
                        # on this chain, so the denominator copies go on
                        # the (idle) scalar engine and the broadcasts are
                        # split by column half so the first normalize
                        # pieces land as early as possible
                        H2 = ICH // 2
                        dnA = ap_.tile([1, ICH], F32, name="dn", bufs=4)
                        nc.scalar.copy(dnA[:], opsA[HD:HD + 1, :])
                        rcA = ap_.tile([1, ICH], F32, name="recip", bufs=4)
                        nc.vector.reciprocal_approx_fast(rcA[:], dnA[:])
                        rbA = ap_.tile([HD, ICH], F32, name="rb", bufs=4)
                        nc.gpsimd.partition_broadcast(rbA[:, 0:H2],
                                                      rcA[:, 0:H2])
                        for jt in range(max(njt - 4, 0), njt):
                            emit_av_b(jt)
                        dnB = ap_.tile([1, ICH], F32, name="dn", bufs=4)
                        nc.scalar.copy(dnB[:], opsB[HD:HD + 1, :])
                        rcB = ap_.tile([1, ICH], F32, name="recip", bufs=4)
                        nc.vector.reciprocal_approx_fast(rcB[:], dnB[:])
                        rbB = ap_.tile([HD, ICH], F32, name="rb", bufs=4)
                        nc.gpsimd.partition_broadcast(rbB[:, 0:H2],
                                                      rcB[:, 0:H2])
                        nc.gpsimd.partition_broadcast(rbA[:, H2:ICH],
                                                      rcA[:, H2:ICH])
                        nc.gpsimd.partition_broadcast(rbB[:, H2:ICH],
                                                      rcB[:, H2:ICH])

                    def norm_mult(split=False, hp=hp, opsA=opsA, opsB=opsB,
                                  rbA=rbA, rbB=rbB):
                        # normalize straight out of PSUM: in0 is PSUM so the
                        # SBUF base-partition pairing rule doesn't apply.
                        # split=True (very last head pair) emits 128-column
                        # pieces so the final out-projection's k=3 matmuls
                        # unblock progressively instead of all at once.
                        pieces = range(4) if split else (slice(None),)
                        for pc in pieces:
                            csl = (slice(pc * P, (pc + 1) * P)
                                   if isinstance(pc, int) else pc)
                            asl = slice(ic * ICH + (csl.start or 0),
                                        ic * ICH + (csl.stop or ICH))
                            for po, ops, rb in ((0, opsA, rbA),
                                                (HD, opsB, rbB)):
                                nc.vector.tensor_tensor(
                                    at[hp][po:po + HD, asl],
                                    ops[0:HD, csl], rb[:, csl],
                                    op=ALU.mult)

                    pending_mults.append(norm_mult)

                while pending_mults:
                    pending_mults.pop()(split=(ic == NIC - 1))
                for f in fillers:
                    f()

            def emit_out_group(mt, nch2):
                pso = sp.tile([P, 512], F32, name="misc", bufs=1)
                for k in range(MT):
                    nc.tensor.matmul(
                        pso[:], at[k][:, mt * P:(mt + 1) * P],
                        wo_sb[:, k, nch2 * 512:(nch2 + 1) * 512],
                        start=(k == 0), stop=(k == MT - 1))
                ob = obp.tile([P, 512], F32, name="ob")
                nc.vector.tensor_copy(ob[:], pso[:])
                nc.sync.dma_start(
                    out_d[mt * P:(mt + 1) * P,
                          nch2 * 512:(nch2 + 1) * 512], ob[:])

            for ic in range(NIC):
                fillers = []
                if ic + 1 < NIC:
                    fillers += [
                        (lambda tt=tt: emit_v_tile(tt))
                        for tt in range(4 * ic + 4, 4 * ic + 8)]
                if ic > 0:
                    fillers += [
                        (lambda mt=mt, n=n: emit_out_group(mt, n))
                        for mt in range(4 * (ic - 1), 4 * ic)
                        for n in range(2)]
                emit_attn_chunk(ic, fillers)

            # final out-projection: all eight groups (mt 12..15 x both
            # halves) live on eight distinct psum banks (four spsum
            # halves, the three opsum slots, and misc) so no slot-reuse
            # WAR can stall the PE. Six groups launch k-major (eighteen
            # ready matmuls cover the last normalize chain); their k=3
            # matmuls follow in mt order, matching the column-split
            # normalize so each unblocks as its at-piece lands. The last
            # two groups sit on the opsum slots of the final head pair
            # and start once its normalize mults have read them. Copies
            # alternate between the now-idle scalar and vector engines.
            groups6 = [(12, 0), (12, 1), (13, 0), (13, 1), (14, 0), (15, 0)]
            slots = []
            for _ in range(2):
                t = sp.tile([P, 2 * ICH], F32, name="spsum", bufs=2)
                slots += [t[:, 0:512], t[:, 512:1024]]
            slots.insert(1, sp.tile([P, ICH], F32, name="opsum", bufs=3))
            slots.insert(3, sp.tile([P, 512], F32, name="misc", bufs=1))
            for k in range(MT - 1):
                for g, (mt, nch2) in enumerate(groups6):
                    nc.tensor.matmul(
                        slots[g], at[k][:, mt * P:(mt + 1) * P],
                        wo_sb[:, k, nch2 * 512:(nch2 + 1) * 512],
                        start=(k == 0), stop=False)
            for g, (mt, nch2) in enumerate(groups6):
                nc.tensor.matmul(
                    slots[g], at[MT - 1][:, mt * P:(mt + 1) * P],
                    wo_sb[:, MT - 1, nch2 * 512:(nch2 + 1) * 512],
                    start=False, stop=True)
            tail2 = [(14, 1), (15, 1)]
            slots2 = [sp.tile([P, ICH], F32, name="opsum", bufs=3)
                      for _ in tail2]
            for k in range(MT):
                for g, (mt, nch2) in enumerate(tail2):
                    nc.tensor.matmul(
                        slots2[g], at[k][:, mt * P:(mt + 1) * P],
                        wo_sb[:, k, nch2 * 512:(nch2 + 1) * 512],
                        start=(k == 0), stop=(k == MT - 1))
            # groups6 copies go on scalar only: a vector copy here would
            # queue in front of the normalize piece-mults on the DVE and
            # delay the tail groups behind them
            for g, (mt, nch2) in enumerate(groups6 + tail2):
                pso = (slots + slots2)[g]
                ob = obp.tile([P, 512], F32, name="ob")
                if g < len(groups6):
                    nc.scalar.copy(ob[:], pso)
                else:
                    nc.vector.tensor_copy(ob[:], pso)
                nc.sync.dma_start(
                    out_d[mt * P:(mt + 1) * P,
                          nch2 * 512:(nch2 + 1) * 512], ob[:])

    nc.compile()
    return nc


def _get_program():
    global _PROGRAM
    if _PROGRAM is None:
        _install_ntff_hook()
        _PROGRAM = _build_program()
    return _PROGRAM


def _make_masks():
    """Multiplicative upper-triangle zero mask [128, 128] for the diagonal
    128x128 block of each S^T tile: entry (j, i) = 1 if j <= i else 0."""
    j = np.arange(P)[:, None]
    i = np.arange(P)[None, :]
    return (j <= i).astype(np.float32)


def make_in_maps(x, Wq, bq, Wk, bk, Wv, bv, Wo, bo):
    import ml_dtypes
    bf16 = ml_dtypes.bfloat16

    def sbl(a, k):
        """[k*128, n] -> SBUF layout [128, k*n] (partition-major runs)."""
        n = a.shape[1]
        return np.ascontiguousarray(
            a.reshape(k, P, n).transpose(1, 0, 2).reshape(P, k * n)
        ).astype(bf16)

    masks = _make_masks()
    in_maps = []
    for c in range(8):
        b, hg = c // 2, c % 2
        sl = slice(hg * QD, (hg + 1) * QD)
        in_maps.append({
            "xT": sbl(np.ascontiguousarray(x[b].T), KT),
            "wq": sbl(Wq[:, sl] * SCALE, KT),
            "wk": sbl(Wk[:, sl], KT),
            "wv": sbl(Wv[:, sl], KT),
            "wo": sbl(Wo[sl, :], MT),
            "bq": np.ascontiguousarray((bq[sl] * SCALE).reshape(MT, P).T),
            "bk": np.ascontiguousarray(bk[sl].reshape(MT, P).T),
            "msk": masks.astype(bf16),
            "idn": np.eye(P, dtype=np.float32).astype(bf16),
            "neg": ((1.0 - masks) * -100000.0).astype(bf16),
        })
    return in_maps


def run(inputs, trace=False):
    from concourse.bass_utils import run_bass_kernel_spmd

    nc = _get_program()
    in_maps = make_in_maps(**inputs)
    res = run_bass_kernel_spmd(nc, in_maps, list(range(8)), trace=trace)
    # softmax rows sum to 1, so the V bias adds bv to every attention
    # output exactly; fold bv @ Wo into the host-side output bias
    bo_eff = inputs["bo"] + inputs["bv"].astype(np.float64) @ \
        inputs["Wo"].astype(np.float64)
    bo_eff = bo_eff.astype(np.float32)
    out = np.empty((B, T, D), dtype=np.float32)
    for b in range(B):
        out[b] = res.results[2 * b]["out"] + res.results[2 * b + 1]["out"] \
            + bo_eff
    return out, res


def kernel(**inputs):
    inputs = {k: np.asarray(v) for k, v in inputs.items()}
    out, _ = run(inputs)
    return out
